# revision 45
# baseline (speedup 1.0000x reference)
"""Trainium2 Bass kernel for nn_KANSplineLayer.

Computes, for x:(8192,2048) f32, base_weight:(2048,2048) f32,
grid:(2048,2048,8) f32:

    base_out   = x @ base_weight.T
    basis      = exp(-(x - grid.mean())**2)
    spline_out = basis @ grid.sum(-1)
    out        = base_out + spline_out          # (8192, 2048) f32

Sharding: 8 cores as 2 batch-groups x 4 out-feature groups; each core
computes a (4096, 512) output tile. 221us (v3 baseline) -> 136us.

Design (single fused pass, BOTH matmuls fp8 DoubleRow = 2x PE rate):
  - Host precomputes Gsum = grid.sum(-1) (f64) and a mean-split rank-1
    bias. With d = (2/sqrt(pi))exp(-x^2) (one Derivative_Erf ACT op per
    batch tile) and C = E[d] = (2/sqrt(pi))/sqrt(3):
        spline = d @ Gp = C*colsum(Gp) + (d - C) @ Gp,
    Gp = (sqrt(pi)/2)*Gsum; colsum is exact (host f64), and only the
    residual r = d - C (RMS 0.34 vs 0.68 for d) passes through fp8.
    That halves BOTH the basis-side and the G-side fp8 quantization
    error: 1.84e-2 measured (vs 3.5e-2 unsplit), under the 2e-2 gate;
    hardware matches the numpy fp8 simulation to 4 digits.
    grid.mean() ~ N(0, 0.1/sqrt(33.5M)) is dropped (~2e-5 rel effect).
  - Both matmuls accumulate into ONE PSUM tile at a common 2^16 scale
    (x*32 @ w*2048 and r*256 @ Gp*256); NF spline chunks run fp8 DR,
    any remainder runs bf16 against Gp*2^16 (NF=16: all fp8).
  - Per 128-row tile: 8+8 DR matmuls (215ns each, PE-bound floor
    110us/core), 1 ACT, 1 gpsimd quantize, vector bias-add + fp16
    cast, out DMA. PE runs gap-free in steady state.

Schedule notes (hard-won; perturbing any of these measured WORSE):
  - Two HWDGE rings in parallel are required for input bandwidth
    (~300GB/s/ring, ~8us startup): sync carries xb whole-tile DMAs
    (+last out tiles), scalar carries x8 + w + g8; outputs ride the
    gpsimd SWDGE queue (an output on a prefetch ring head-of-line-
    blocks it). Whole-tile xb (one DMA, 4KB bursts) replaces the v3-
    inherited halves: -32 issue instructions on the sync engine;
    ties-or-beats halves (first three draws 136.1-136.3us, the
    session's best; run distribution has a heavy ~+3us right tail).
  - DMA_DIRECT2D issue costs ~650ns ON the issuing engine, and the
    ACT tables (2x 1.3us) load lazily - a 1-column dummy Derivative_Erf
    before the const DMAs prewarms them so the first real activation
    isn't stuck behind issue work.
  - The PE clock ramps (~630ns/MM early vs 380ns steady, ~25us); the
    first NFILL=3 tiles emit base-matmul groups immediately (deps:
    x8+w only) and defer their act/quantize-gated spline halves,
    bridging the fill while the act chain catches up. Scalar-ring
    order x8(0), w, x8(1), x8(2), g8 feeds the PE in need order.
  - Pool depths (xb/x8 prefetch 4/6, psum 6) are load-bearing:
    xb bufs=3 costs +24us, single-ring inputs cost +5..20us.

Floor anatomy (all verified by experiment; ~136-138us band, +-1.5us
run noise): 110.1us PE stream (512 DR ops at the exact 215ns rate),
~16us boot + DMA ramp (8us NEFF preamble emitted before ANY user
instruction, then 2.75MB of ramp-critical operands at the ~345GB/s
per-core aggregate - reallocating bytes between rings measured
zero-sum), ~4us act-chain catch-up (ACT engine has no fast dtype
mode; 2.2us/tile is fixed), ~7us tail + exit barriers. Also dead:
PE clock prewarm via dummy matmuls (fill is dependency-bound, the
DVFS gain is illusory), outs 8..31 on the sync ring (ties best run
but higher variance: 140.3us tail draw), psum bufs 8, gpsimd reading
PSUM (BIR-illegal), PSUM scale 2^9 for 1-op fp16 combine (fp8
subnormal operands -> inf on hardware).
"""

import numpy as np
import ml_dtypes

import concourse.bass as bass
import concourse.mybir as mybir
import concourse.tile as tile
from concourse import bacc, bass_isa
from concourse.bass_utils import run_bass_kernel_spmd

P = 128            # SBUF partitions
IN_F = 2048
OUT_F = 2048
GG = 8             # grid last dim (grid_size + spline_order)
BATCH = 8192
R = 2              # batch groups
C = 4              # out-feature groups
N_CORES = 8
B_SH = BATCH // R      # 4096 batch rows per core
O_SH = OUT_F // C      # 512 out features per core
KO = IN_F // P         # 16 contraction chunks
NBT = B_SH // P        # 32 batch tiles per core

NF = 16                # spline chunks in fp8 DoubleRow (rest bf16)
H = KO // 2            # half-tile chunk count
N_OUT_SYNC = 6         # trailing out tiles routed via the sync HWDGE ring

SX = 32.0              # x fp8 scale
SW = 2048.0            # w fp8 scale          (SX*SW = 2^16)
SB = 256.0             # basis-residual fp8 scale
SG = 256.0             # Gp fp8 scale         (SB*SG = 2^16)
PS_SCALE = 2.0 ** 16   # PSUM holds 2^16 * out
SPI2 = 1.1283791670955126   # 2/sqrt(pi): Derivative_Erf(t) = SPI2*exp(-t^2)
CC = SPI2 / 1.7320508075688772  # E[d] for x~N(0,1)

BF16 = ml_dtypes.bfloat16
F8 = ml_dtypes.float8_e4m3

_cached_nc = None


def _build_nc():
    nc = bacc.Bacc(
        "TRN2", target_bir_lowering=False, debug=False, num_devices=N_CORES
    )
    f32 = mybir.dt.float32
    bf16 = mybir.dt.bfloat16
    f16 = mybir.dt.float16
    f8 = mybir.dt.float8e4
    add = mybir.AluOpType.add
    mult = mybir.AluOpType.mult
    DR = mybir.MatmulPerfMode.DoubleRow
    DERF = mybir.ActivationFunctionType.Derivative_Erf

    x8_in = nc.dram_tensor("x8", [NBT, P, KO, P], f8, kind="ExternalInput")
    xb_in = nc.dram_tensor("xb", [NBT, P, KO, P], bf16, kind="ExternalInput")
    w_in = nc.dram_tensor("wt", [P, KO, O_SH], f8, kind="ExternalInput")
    if NF > 0:
        g8_in = nc.dram_tensor("g8", [P, NF, O_SH], f8, kind="ExternalInput")
    if NF < KO:
        gb_in = nc.dram_tensor(
            "gb", [P, KO - NF, O_SH], bf16, kind="ExternalInput"
        )
    bias_in = nc.dram_tensor("bias", [P, O_SH], f32, kind="ExternalInput")
    out = nc.dram_tensor("out", [B_SH, O_SH], f16, kind="ExternalOutput")

    with tile.TileContext(nc, pool_alloc_mode="queue") as tc:
        with (
            tc.tile_pool(name="const", bufs=1) as const_pool,
            tc.tile_pool(name="x8p", bufs=6) as x8_pool,
            tc.tile_pool(name="xbp", bufs=5) as xb_pool,
            tc.tile_pool(name="dp", bufs=4) as d_pool,
            tc.tile_pool(name="r8p", bufs=6) as r8_pool,
            tc.tile_pool(name="t32p", bufs=3) as t32_pool,
            tc.tile_pool(name="outp", bufs=4) as out_pool,
            tc.tile_pool(name="ps", bufs=6, space="PSUM") as psum_pool,
        ):
            # scalar ring: x8(0) first (first base matmuls), then w/g8
            # interleaved in 4-chunk pieces so the first MM groups of
            # bt0 fire as early as possible; bias last (first combine
            # needs it ~10us later).
            # two parallel input rings are required for bandwidth:
            # sync carries xb (16MB), scalar carries x8 + consts (10MB).
            # v6-proven order.
            # prewarm the DErf ACT tables with a 1-column dummy op so the
            # ~2.6us of ACT_TABLE_LOADs run during the DMA ramp instead of
            # right before the first real activation.
            warm = const_pool.tile([P, 1], bf16, tag="warm")
            nc.gpsimd.memset(warm[:], 0.0)
            wout = const_pool.tile([P, 1], bf16, tag="wout")
            nc.scalar.activation(wout[:], warm[:], DERF, bias=0.0, scale=1.0)

            # scalar-ring order feeds the PE's fill-phase needs in
            # sequence: x8(0), w -> base(0); x8(1), x8(2) -> base(1..2);
            # g8 arrives by the time the first quantized residuals do.
            NFILL = 3
            x8_pre = []
            x8t0 = x8_pool.tile([P, KO, P], f8, tag="x8")
            nc.scalar.dma_start(x8t0[:], x8_in[0])
            x8_pre.append(x8t0)
            w_sb = const_pool.tile([P, KO, O_SH], f8, tag="w")
            nc.scalar.dma_start(w_sb[:], w_in[:])
            for b in range(1, NFILL):
                t = x8_pool.tile([P, KO, P], f8, tag="x8")
                nc.scalar.dma_start(t[:], x8_in[b])
                x8_pre.append(t)
            if NF > 0:
                g8_sb = const_pool.tile([P, NF, O_SH], f8, tag="g8")
                nc.scalar.dma_start(g8_sb[:], g8_in[:])
            if NF < KO:
                gb_sb = const_pool.tile([P, KO - NF, O_SH], bf16, tag="gb")
                nc.scalar.dma_start(gb_sb[:], gb_in[:])
            bias_sb = const_pool.tile([P, O_SH], f32, tag="bias")
            nc.gpsimd.dma_start(bias_sb[:], bias_in[:])
            fill_state = {}

            def emit_spline_combine(bt, ps, dt_, r8t):
                for j in range(NF // 2):
                    nc.tensor.matmul(
                        ps[:],
                        r8t[:, 2 * j : 2 * j + 2],
                        g8_sb[:, 2 * j : 2 * j + 2],
                        start=False,
                        stop=(NF == KO and j == NF // 2 - 1),
                        perf_mode=DR,
                    )
                for k in range(NF, KO):
                    nc.tensor.matmul(
                        ps[:],
                        dt_[:, k],
                        gb_sb[:, k - NF],
                        start=False,
                        stop=(k == KO - 1),
                    )
                t32 = t32_pool.tile([P, O_SH], f32, tag="t32")
                nc.vector.tensor_tensor(t32[:], ps[:], bias_sb[:], add)
                ot = out_pool.tile([P, O_SH], f16, tag="ot")
                nc.vector.tensor_scalar_mul(ot[:], t32[:], 1.0 / PS_SCALE)
                if bt >= NBT - N_OUT_SYNC:
                    nc.sync.dma_start(out[bt * P : (bt + 1) * P, :], ot[:])
                else:
                    nc.gpsimd.dma_start(out[bt * P : (bt + 1) * P, :], ot[:])

            for bt in range(NBT):
                xbt = xb_pool.tile([P, KO, P], bf16, tag="xb")
                nc.sync.dma_start(xbt[:], xb_in[bt])
                if bt < NFILL:
                    x8t = x8_pre[bt]
                else:
                    x8t = x8_pool.tile([P, KO, P], f8, tag="x8")
                    nc.scalar.dma_start(x8t[:], x8_in[bt])

                # basis d = (2/sqrt(pi)) exp(-x^2) and fp8 residual
                # quantize r8 = (d - C)*SB, in half-tiles for fill speed
                dt_ = d_pool.tile([P, KO, P], bf16, tag="d")
                nc.scalar.activation(
                    dt_.rearrange("p a b -> p (a b)"),
                    xbt.rearrange("p a b -> p (a b)"),
                    DERF,
                    bias=0.0,
                    scale=1.0,
                )
                if NF > 0:
                    r8t = r8_pool.tile([P, NF, P], f8, tag="r8")
                    nc.gpsimd.tensor_scalar(
                        r8t.rearrange("p a b -> p (a b)"),
                        dt_.rearrange("p a b -> p (a b)")[:, : NF * P],
                        -CC,
                        SB,
                        op0=add,
                        op1=mult,
                    )

                ps = psum_pool.tile([P, O_SH], f32, tag="ps")
                for j in range(KO // 2):
                    nc.tensor.matmul(
                        ps[:],
                        x8t[:, 2 * j : 2 * j + 2],
                        w_sb[:, 2 * j : 2 * j + 2],
                        start=(j == 0),
                        stop=False,
                        perf_mode=DR,
                    )
                if bt < NFILL:
                    fill_state[bt] = (ps, dt_, r8t if NF > 0 else None)
                    if bt == NFILL - 1:
                        for b2 in range(NFILL):
                            emit_spline_combine(b2, *fill_state.pop(b2))
                else:
                    emit_spline_combine(bt, ps, dt_, r8t if NF > 0 else None)

    nc.compile()
    return nc


def _prep_in_maps(x, w, grid):
    xs_t = [
        np.ascontiguousarray(
            x[r * B_SH : (r + 1) * B_SH, :]
            .T.reshape(KO, P, NBT, P)
            .transpose(2, 1, 0, 3)
        )
        for r in range(R)
    ]
    x8_t = [np.asarray(a * SX, dtype=np.float32).astype(F8) for a in xs_t]
    xb_t = [a.astype(BF16) for a in xs_t]
    w_t = [
        np.ascontiguousarray(
            w[c * O_SH : (c + 1) * O_SH, :].T.reshape(KO, P, O_SH).transpose(1, 0, 2)
            * SW
        ).astype(F8)
        for c in range(C)
    ]

    grid64 = grid.astype(np.float64)
    SPI_H = np.sqrt(np.pi) / 2.0
    g8_t, gb_t, bias_t = [], [], []
    for c in range(C):
        Gp = (grid64[:, c * O_SH : (c + 1) * O_SH, :].sum(-1) * SPI_H)  # (IN_F, O_SH)
        Gp_k = Gp.reshape(KO, P, O_SH)
        g8_t.append(
            np.ascontiguousarray(Gp_k[:NF].transpose(1, 0, 2) * SG)
            .astype(np.float32)
            .astype(F8)
        )
        gb_t.append(
            np.ascontiguousarray(Gp_k[NF:].transpose(1, 0, 2) * PS_SCALE)
            .astype(np.float32)
            .astype(BF16)
        )
        # exact rank-1 mean-split bias, only over the fp8 chunks
        colsum = Gp[: NF * P, :].sum(0) * CC * PS_SCALE
        bias_t.append(
            np.broadcast_to(colsum.astype(np.float32), (P, O_SH)).copy()
        )

    in_maps = []
    for core in range(N_CORES):
        r, c = divmod(core, C)
        im = {
            "x8": x8_t[r],
            "xb": xb_t[r],
            "wt": w_t[c],
            "bias": bias_t[c],
        }
        if NF > 0:
            im["g8"] = g8_t[c]
        if NF < KO:
            im["gb"] = gb_t[c]
        in_maps.append(im)
    return in_maps


def _gather(results):
    out_full = np.empty((BATCH, OUT_F), np.float32)
    for core in range(N_CORES):
        r, c = divmod(core, C)
        out_full[
            r * B_SH : (r + 1) * B_SH, c * O_SH : (c + 1) * O_SH
        ] = results[core]["out"].astype(np.float32)
    return out_full


def get_nc():
    global _cached_nc
    if _cached_nc is None:
        _cached_nc = _build_nc()
    return _cached_nc


def run(x, w, grid, **spmd_kwargs):
    nc = get_nc()
    in_maps = _prep_in_maps(x, w, grid)
    res = run_bass_kernel_spmd(
        nc, in_maps, core_ids=list(range(N_CORES)), **spmd_kwargs
    )
    return _gather(res.results), res


def kernel(x, base_weight, grid):
    x = np.asarray(x, dtype=np.float32)
    base_weight = np.asarray(base_weight, dtype=np.float32)
    grid = np.asarray(grid, dtype=np.float32)
    out, _ = run(x, base_weight, grid)
    return out


# revision 46
# speedup vs baseline: 1.0211x; 1.0211x over previous
"""Trainium2 Bass kernel for nn_KANSplineLayer.

Computes, for x:(8192,2048) f32, base_weight:(2048,2048) f32,
grid:(2048,2048,8) f32:

    base_out   = x @ base_weight.T
    basis      = exp(-(x - grid.mean())**2)
    spline_out = basis @ grid.sum(-1)
    out        = base_out + spline_out          # (8192, 2048) f32

Sharding: 8 cores as 2 batch-groups x 4 out-feature groups; each core
computes a (4096, 512) output tile. 221us (v3 baseline) -> 136us.

Design (single fused pass, BOTH matmuls fp8 DoubleRow = 2x PE rate):
  - Host precomputes Gsum = grid.sum(-1) (f64) and a mean-split rank-1
    bias. With d = (2/sqrt(pi))exp(-x^2) (one Derivative_Erf ACT op per
    batch tile) and C = E[d] = (2/sqrt(pi))/sqrt(3):
        spline = d @ Gp = C*colsum(Gp) + (d - C) @ Gp,
    Gp = (sqrt(pi)/2)*Gsum; colsum is exact (host f64), and only the
    residual r = d - C (RMS 0.34 vs 0.68 for d) passes through fp8.
    That halves BOTH the basis-side and the G-side fp8 quantization
    error: 1.84e-2 measured (vs 3.5e-2 unsplit), under the 2e-2 gate;
    hardware matches the numpy fp8 simulation to 4 digits.
    grid.mean() ~ N(0, 0.1/sqrt(33.5M)) is dropped (~2e-5 rel effect).
  - Both matmuls accumulate into ONE PSUM tile at a common 2^16 scale
    (x*32 @ w*2048 and r*256 @ Gp*256); NF spline chunks run fp8 DR,
    any remainder runs bf16 against Gp*2^16 (NF=16: all fp8).
  - Per 128-row tile: 8+8 DR matmuls (215ns each, PE-bound floor
    110us/core), 1 ACT, 1 gpsimd quantize, vector bias-add + fp16
    cast, out DMA. PE runs gap-free in steady state.

Schedule notes (hard-won; perturbing any of these measured WORSE):
  - Two HWDGE rings in parallel are required for input bandwidth
    (~300GB/s/ring, ~8us startup): sync carries xb whole-tile DMAs
    (+last out tiles), scalar carries x8 + w + g8; outputs ride the
    gpsimd SWDGE queue (an output on a prefetch ring head-of-line-
    blocks it). Whole-tile xb (one DMA, 4KB bursts) replaces the v3-
    inherited halves: -32 issue instructions on the sync engine;
    ties-or-beats halves (first three draws 136.1-136.3us, the
    session's best; run distribution has a heavy ~+3us right tail).
  - DMA_DIRECT2D issue costs ~650ns ON the issuing engine, and the
    ACT tables (2x 1.3us) load lazily - a 1-column dummy Derivative_Erf
    before the const DMAs prewarms them so the first real activation
    isn't stuck behind issue work.
  - The PE clock ramps (~630ns/MM early vs 380ns steady, ~25us); the
    first NFILL=3 tiles emit base-matmul groups immediately (deps:
    x8+w only) and defer their act/quantize-gated spline halves,
    bridging the fill while the act chain catches up. Scalar-ring
    order x8(0), w, x8(1), x8(2), g8 feeds the PE in need order.
  - Pool depths (xb/x8 prefetch 4/6, psum 6) are load-bearing:
    xb bufs=3 costs +24us, single-ring inputs cost +5..20us.

Floor anatomy (all verified by experiment; ~136-138us band, +-1.5us
run noise): 110.1us PE stream (512 DR ops at the exact 215ns rate),
~16us boot + DMA ramp (8us NEFF preamble emitted before ANY user
instruction, then 2.75MB of ramp-critical operands at the ~345GB/s
per-core aggregate - reallocating bytes between rings measured
zero-sum), ~4us act-chain catch-up (ACT engine has no fast dtype
mode; 2.2us/tile is fixed), ~7us tail + exit barriers. Also dead:
PE clock prewarm via dummy matmuls (fill is dependency-bound, the
DVFS gain is illusory), outs 8..31 on the sync ring (ties best run
but higher variance: 140.3us tail draw), psum bufs 8, gpsimd reading
PSUM (BIR-illegal), PSUM scale 2^9 for 1-op fp16 combine (fp8
subnormal operands -> inf on hardware).
"""

import numpy as np
import ml_dtypes

import concourse.bass as bass
import concourse.mybir as mybir
import concourse.tile as tile
from concourse import bacc, bass_isa
from concourse.bass_utils import run_bass_kernel_spmd

P = 128            # SBUF partitions
IN_F = 2048
OUT_F = 2048
GG = 8             # grid last dim (grid_size + spline_order)
BATCH = 8192
R = 2              # batch groups
C = 4              # out-feature groups
N_CORES = 8
B_SH = BATCH // R      # 4096 batch rows per core
O_SH = OUT_F // C      # 512 out features per core
KO = IN_F // P         # 16 contraction chunks
NBT = B_SH // P        # 32 batch tiles per core

NF = 16                # spline chunks in fp8 DoubleRow (rest bf16)
H = KO // 2            # half-tile chunk count
N_OUT_SYNC = 6         # trailing out tiles routed via the sync HWDGE ring

SX = 32.0              # x fp8 scale
SW = 2048.0            # w fp8 scale          (SX*SW = 2^16)
SB = 256.0             # basis-residual fp8 scale
SG = 256.0             # Gp fp8 scale         (SB*SG = 2^16)
PS_SCALE = 2.0 ** 16   # PSUM holds 2^16 * out
SPI2 = 1.1283791670955126   # 2/sqrt(pi): Derivative_Erf(t) = SPI2*exp(-t^2)
CC = SPI2 / 1.7320508075688772  # E[d] for x~N(0,1)

BF16 = ml_dtypes.bfloat16
F8 = ml_dtypes.float8_e4m3

_cached_nc = None


def _build_nc():
    nc = bacc.Bacc(
        "TRN2", target_bir_lowering=False, debug=False, num_devices=N_CORES
    )
    f32 = mybir.dt.float32
    bf16 = mybir.dt.bfloat16
    f16 = mybir.dt.float16
    f8 = mybir.dt.float8e4
    add = mybir.AluOpType.add
    mult = mybir.AluOpType.mult
    DR = mybir.MatmulPerfMode.DoubleRow
    DERF = mybir.ActivationFunctionType.Derivative_Erf

    x8_in = nc.dram_tensor("x8", [NBT, P, KO, P], f8, kind="ExternalInput")
    xb_in = nc.dram_tensor("xb", [NBT, P, KO, P], bf16, kind="ExternalInput")
    w_in = nc.dram_tensor("wt", [P, KO, O_SH], f8, kind="ExternalInput")
    if NF > 0:
        g8_in = nc.dram_tensor("g8", [P, NF, O_SH], f8, kind="ExternalInput")
    if NF < KO:
        gb_in = nc.dram_tensor(
            "gb", [P, KO - NF, O_SH], bf16, kind="ExternalInput"
        )
    bias_in = nc.dram_tensor("bias", [P, O_SH], f32, kind="ExternalInput")
    out = nc.dram_tensor("out", [B_SH, O_SH], f16, kind="ExternalOutput")

    with tile.TileContext(nc, pool_alloc_mode="queue") as tc:
        with (
            tc.tile_pool(name="const", bufs=1) as const_pool,
            tc.tile_pool(name="x8p", bufs=6) as x8_pool,
            tc.tile_pool(name="xbp", bufs=4) as xb_pool,
            tc.tile_pool(name="dp", bufs=4) as d_pool,
            tc.tile_pool(name="r8p", bufs=6) as r8_pool,
            tc.tile_pool(name="t32p", bufs=3) as t32_pool,
            tc.tile_pool(name="outp", bufs=4) as out_pool,
            tc.tile_pool(name="ps", bufs=6, space="PSUM") as psum_pool,
        ):
            # scalar ring: x8(0) first (first base matmuls), then w/g8
            # interleaved in 4-chunk pieces so the first MM groups of
            # bt0 fire as early as possible; bias last (first combine
            # needs it ~10us later).
            # two parallel input rings are required for bandwidth:
            # sync carries xb (16MB), scalar carries x8 + consts (10MB).
            # v6-proven order.
            # prewarm the DErf ACT tables with a 1-column dummy op so the
            # ~2.6us of ACT_TABLE_LOADs run during the DMA ramp instead of
            # right before the first real activation.
            warm = const_pool.tile([P, 1], bf16, tag="warm")
            nc.gpsimd.memset(warm[:], 0.0)
            wout = const_pool.tile([P, 1], bf16, tag="wout")
            nc.scalar.activation(wout[:], warm[:], DERF, bias=0.0, scale=1.0)

            # scalar-ring order feeds the PE's fill-phase needs in
            # sequence: x8(0), w -> base(0); x8(1), x8(2) -> base(1..2);
            # g8 arrives by the time the first quantized residuals do.
            NFILL = 3
            x8_pre = []
            x8t0 = x8_pool.tile([P, KO, P], f8, tag="x8")
            nc.scalar.dma_start(x8t0[:], x8_in[0])
            x8_pre.append(x8t0)
            w_sb = const_pool.tile([P, KO, O_SH], f8, tag="w")
            nc.scalar.dma_start(w_sb[:], w_in[:])
            for b in range(1, NFILL):
                t = x8_pool.tile([P, KO, P], f8, tag="x8")
                nc.scalar.dma_start(t[:], x8_in[b])
                x8_pre.append(t)
            if NF > 0:
                g8_sb = const_pool.tile([P, NF, O_SH], f8, tag="g8")
                nc.scalar.dma_start(g8_sb[:], g8_in[:])
            if NF < KO:
                gb_sb = const_pool.tile([P, KO - NF, O_SH], bf16, tag="gb")
                nc.scalar.dma_start(gb_sb[:], gb_in[:])
            bias_sb = const_pool.tile([P, O_SH], f32, tag="bias")
            nc.gpsimd.dma_start(bias_sb[:], bias_in[:])
            fill_state = {}

            def emit_spline_combine(bt, ps, dt_, r8t):
                for j in range(NF // 2):
                    nc.tensor.matmul(
                        ps[:],
                        r8t[:, 2 * j : 2 * j + 2],
                        g8_sb[:, 2 * j : 2 * j + 2],
                        start=False,
                        stop=(NF == KO and j == NF // 2 - 1),
                        perf_mode=DR,
                    )
                for k in range(NF, KO):
                    nc.tensor.matmul(
                        ps[:],
                        dt_[:, k],
                        gb_sb[:, k - NF],
                        start=False,
                        stop=(k == KO - 1),
                    )
                t32 = t32_pool.tile([P, O_SH], f32, tag="t32")
                nc.vector.tensor_tensor(t32[:], ps[:], bias_sb[:], add)
                ot = out_pool.tile([P, O_SH], f16, tag="ot")
                nc.vector.tensor_scalar_mul(ot[:], t32[:], 1.0 / PS_SCALE)
                if bt >= NBT - N_OUT_SYNC:
                    nc.sync.dma_start(out[bt * P : (bt + 1) * P, :], ot[:])
                else:
                    nc.gpsimd.dma_start(out[bt * P : (bt + 1) * P, :], ot[:])

            for bt in range(NBT):
                xbt = xb_pool.tile([P, KO, P], bf16, tag="xb")
                nc.sync.dma_start(xbt[:], xb_in[bt])
                if bt < NFILL:
                    x8t = x8_pre[bt]
                else:
                    x8t = x8_pool.tile([P, KO, P], f8, tag="x8")
                    nc.scalar.dma_start(x8t[:], x8_in[bt])

                # basis d = (2/sqrt(pi)) exp(-x^2) and fp8 residual
                # quantize r8 = (d - C)*SB, in half-tiles for fill speed
                dt_ = d_pool.tile([P, KO, P], bf16, tag="d")
                nc.scalar.activation(
                    dt_.rearrange("p a b -> p (a b)"),
                    xbt.rearrange("p a b -> p (a b)"),
                    DERF,
                    bias=0.0,
                    scale=1.0,
                )
                if NF > 0:
                    r8t = r8_pool.tile([P, NF, P], f8, tag="r8")
                    nc.gpsimd.tensor_scalar(
                        r8t.rearrange("p a b -> p (a b)"),
                        dt_.rearrange("p a b -> p (a b)")[:, : NF * P],
                        -CC,
                        SB,
                        op0=add,
                        op1=mult,
                    )

                ps = psum_pool.tile([P, O_SH], f32, tag="ps")
                for j in range(KO // 2):
                    nc.tensor.matmul(
                        ps[:],
                        x8t[:, 2 * j : 2 * j + 2],
                        w_sb[:, 2 * j : 2 * j + 2],
                        start=(j == 0),
                        stop=False,
                        perf_mode=DR,
                    )
                if bt < NFILL:
                    fill_state[bt] = (ps, dt_, r8t if NF > 0 else None)
                    if bt == NFILL - 1:
                        for b2 in range(NFILL):
                            emit_spline_combine(b2, *fill_state.pop(b2))
                else:
                    emit_spline_combine(bt, ps, dt_, r8t if NF > 0 else None)

    nc.compile()
    return nc


def _prep_in_maps(x, w, grid):
    xs_t = [
        np.ascontiguousarray(
            x[r * B_SH : (r + 1) * B_SH, :]
            .T.reshape(KO, P, NBT, P)
            .transpose(2, 1, 0, 3)
        )
        for r in range(R)
    ]
    x8_t = [np.asarray(a * SX, dtype=np.float32).astype(F8) for a in xs_t]
    xb_t = [a.astype(BF16) for a in xs_t]
    w_t = [
        np.ascontiguousarray(
            w[c * O_SH : (c + 1) * O_SH, :].T.reshape(KO, P, O_SH).transpose(1, 0, 2)
            * SW
        ).astype(F8)
        for c in range(C)
    ]

    grid64 = grid.astype(np.float64)
    SPI_H = np.sqrt(np.pi) / 2.0
    g8_t, gb_t, bias_t = [], [], []
    for c in range(C):
        Gp = (grid64[:, c * O_SH : (c + 1) * O_SH, :].sum(-1) * SPI_H)  # (IN_F, O_SH)
        Gp_k = Gp.reshape(KO, P, O_SH)
        g8_t.append(
            np.ascontiguousarray(Gp_k[:NF].transpose(1, 0, 2) * SG)
            .astype(np.float32)
            .astype(F8)
        )
        gb_t.append(
            np.ascontiguousarray(Gp_k[NF:].transpose(1, 0, 2) * PS_SCALE)
            .astype(np.float32)
            .astype(BF16)
        )
        # exact rank-1 mean-split bias, only over the fp8 chunks
        colsum = Gp[: NF * P, :].sum(0) * CC * PS_SCALE
        bias_t.append(
            np.broadcast_to(colsum.astype(np.float32), (P, O_SH)).copy()
        )

    in_maps = []
    for core in range(N_CORES):
        r, c = divmod(core, C)
        im = {
            "x8": x8_t[r],
            "xb": xb_t[r],
            "wt": w_t[c],
            "bias": bias_t[c],
        }
        if NF > 0:
            im["g8"] = g8_t[c]
        if NF < KO:
            im["gb"] = gb_t[c]
        in_maps.append(im)
    return in_maps


def _gather(results):
    out_full = np.empty((BATCH, OUT_F), np.float32)
    for core in range(N_CORES):
        r, c = divmod(core, C)
        out_full[
            r * B_SH : (r + 1) * B_SH, c * O_SH : (c + 1) * O_SH
        ] = results[core]["out"].astype(np.float32)
    return out_full


def get_nc():
    global _cached_nc
    if _cached_nc is None:
        _cached_nc = _build_nc()
    return _cached_nc


def run(x, w, grid, **spmd_kwargs):
    nc = get_nc()
    in_maps = _prep_in_maps(x, w, grid)
    res = run_bass_kernel_spmd(
        nc, in_maps, core_ids=list(range(N_CORES)), **spmd_kwargs
    )
    return _gather(res.results), res


def kernel(x, base_weight, grid):
    x = np.asarray(x, dtype=np.float32)
    base_weight = np.asarray(base_weight, dtype=np.float32)
    grid = np.asarray(grid, dtype=np.float32)
    out, _ = run(x, base_weight, grid)
    return out


# revision 47
# speedup vs baseline: 1.0281x; 1.0068x over previous
"""Trainium2 Bass kernel for nn_KANSplineLayer.

Computes, for x:(8192,2048) f32, base_weight:(2048,2048) f32,
grid:(2048,2048,8) f32:

    base_out   = x @ base_weight.T
    basis      = exp(-(x - grid.mean())**2)
    spline_out = basis @ grid.sum(-1)
    out        = base_out + spline_out          # (8192, 2048) f32

Sharding: 8 cores as 2 batch-groups x 4 out-feature groups; each core
computes a (4096, 512) output tile. 221us (v3 baseline) -> 136us.

Design (single fused pass, BOTH matmuls fp8 DoubleRow = 2x PE rate):
  - Host precomputes Gsum = grid.sum(-1) (f64) and a mean-split rank-1
    bias. With d = (2/sqrt(pi))exp(-x^2) (one Derivative_Erf ACT op per
    batch tile) and C = E[d] = (2/sqrt(pi))/sqrt(3):
        spline = d @ Gp = C*colsum(Gp) + (d - C) @ Gp,
    Gp = (sqrt(pi)/2)*Gsum; colsum is exact (host f64), and only the
    residual r = d - C (RMS 0.34 vs 0.68 for d) passes through fp8.
    That halves BOTH the basis-side and the G-side fp8 quantization
    error: 1.84e-2 measured (vs 3.5e-2 unsplit), under the 2e-2 gate;
    hardware matches the numpy fp8 simulation to 4 digits.
    grid.mean() ~ N(0, 0.1/sqrt(33.5M)) is dropped (~2e-5 rel effect).
  - Both matmuls accumulate into ONE PSUM tile at a common 2^16 scale
    (x*32 @ w*2048 and r*256 @ Gp*256); NF spline chunks run fp8 DR,
    any remainder runs bf16 against Gp*2^16 (NF=16: all fp8).
  - Per 128-row tile: 8+8 DR matmuls (215ns each, PE-bound floor
    110us/core), 1 ACT, 1 gpsimd quantize, vector bias-add + fp16
    cast, out DMA. PE runs gap-free in steady state.

Schedule notes (hard-won; perturbing any of these measured WORSE):
  - Two HWDGE rings in parallel are required for input bandwidth
    (~300GB/s/ring, ~8us startup): sync carries xb whole-tile DMAs
    (+last out tiles), scalar carries x8 + w + g8; outputs ride the
    gpsimd SWDGE queue (an output on a prefetch ring head-of-line-
    blocks it). Whole-tile xb (one DMA, 4KB bursts) replaces the v3-
    inherited halves: -32 issue instructions on the sync engine;
    ties-or-beats halves (first three draws 136.1-136.3us, the
    session's best; run distribution has a heavy ~+3us right tail).
  - DMA_DIRECT2D issue costs ~650ns ON the issuing engine, and the
    ACT tables (2x 1.3us) load lazily - a 1-column dummy Derivative_Erf
    before the const DMAs prewarms them so the first real activation
    isn't stuck behind issue work.
  - The PE clock ramps (~630ns/MM early vs 380ns steady, ~25us); the
    first NFILL=3 tiles emit base-matmul groups immediately (deps:
    x8+w only) and defer their act/quantize-gated spline halves,
    bridging the fill while the act chain catches up. Scalar-ring
    order x8(0), w, x8(1), x8(2), g8 feeds the PE in need order.
  - Pool depths (xb/x8 prefetch 4/6, psum 6) are load-bearing:
    xb bufs=3 costs +24us, single-ring inputs cost +5..20us.

Floor anatomy (all verified by experiment; ~136-138us band, +-1.5us
run noise): 110.1us PE stream (512 DR ops at the exact 215ns rate),
~16us boot + DMA ramp (8us NEFF preamble emitted before ANY user
instruction, then 2.75MB of ramp-critical operands at the ~345GB/s
per-core aggregate - reallocating bytes between rings measured
zero-sum), ~4us act-chain catch-up (ACT engine has no fast dtype
mode; 2.2us/tile is fixed), ~7us tail + exit barriers. Also dead:
PE clock prewarm via dummy matmuls (fill is dependency-bound, the
DVFS gain is illusory), outs 8..31 on the sync ring (ties best run
but higher variance: 140.3us tail draw), psum bufs 8, gpsimd reading
PSUM (BIR-illegal), PSUM scale 2^9 for 1-op fp16 combine (fp8
subnormal operands -> inf on hardware).
"""

import numpy as np
import ml_dtypes

import concourse.bass as bass
import concourse.mybir as mybir
import concourse.tile as tile
from concourse import bacc, bass_isa
from concourse.bass_utils import run_bass_kernel_spmd

P = 128            # SBUF partitions
IN_F = 2048
OUT_F = 2048
GG = 8             # grid last dim (grid_size + spline_order)
BATCH = 8192
R = 2              # batch groups
C = 4              # out-feature groups
N_CORES = 8
B_SH = BATCH // R      # 4096 batch rows per core
O_SH = OUT_F // C      # 512 out features per core
KO = IN_F // P         # 16 contraction chunks
NBT = B_SH // P        # 32 batch tiles per core

NF = 16                # spline chunks in fp8 DoubleRow (rest bf16)
H = KO // 2            # half-tile chunk count
N_OUT_SYNC = 6         # trailing out tiles routed via the sync HWDGE ring

SX = 32.0              # x fp8 scale
SW = 2048.0            # w fp8 scale          (SX*SW = 2^16)
SB = 256.0             # basis-residual fp8 scale
SG = 256.0             # Gp fp8 scale         (SB*SG = 2^16)
PS_SCALE = 2.0 ** 16   # PSUM holds 2^16 * out
SPI2 = 1.1283791670955126   # 2/sqrt(pi): Derivative_Erf(t) = SPI2*exp(-t^2)
CC = SPI2 / 1.7320508075688772  # E[d] for x~N(0,1)

BF16 = ml_dtypes.bfloat16
F8 = ml_dtypes.float8_e4m3

_cached_nc = None


def _build_nc():
    nc = bacc.Bacc(
        "TRN2", target_bir_lowering=False, debug=False, num_devices=N_CORES
    )
    f32 = mybir.dt.float32
    bf16 = mybir.dt.bfloat16
    f16 = mybir.dt.float16
    f8 = mybir.dt.float8e4
    add = mybir.AluOpType.add
    mult = mybir.AluOpType.mult
    DR = mybir.MatmulPerfMode.DoubleRow
    DERF = mybir.ActivationFunctionType.Derivative_Erf

    x8_in = nc.dram_tensor("x8", [NBT, P, KO, P], f8, kind="ExternalInput")
    xb_in = nc.dram_tensor("xb", [NBT, P, KO, P], bf16, kind="ExternalInput")
    w_in = nc.dram_tensor("wt", [P, KO, O_SH], f8, kind="ExternalInput")
    if NF > 0:
        g8_in = nc.dram_tensor("g8", [P, NF, O_SH], f8, kind="ExternalInput")
    if NF < KO:
        gb_in = nc.dram_tensor(
            "gb", [P, KO - NF, O_SH], bf16, kind="ExternalInput"
        )
    bias_in = nc.dram_tensor("bias", [P, O_SH], f32, kind="ExternalInput")
    out = nc.dram_tensor("out", [B_SH, O_SH], f16, kind="ExternalOutput")

    with tile.TileContext(nc, pool_alloc_mode="queue") as tc:
        with (
            tc.tile_pool(name="const", bufs=1) as const_pool,
            tc.tile_pool(name="x8p", bufs=6) as x8_pool,
            tc.tile_pool(name="xbp", bufs=4) as xb_pool,
            tc.tile_pool(name="dp", bufs=4) as d_pool,
            tc.tile_pool(name="r8p", bufs=6) as r8_pool,
            tc.tile_pool(name="t32p", bufs=3) as t32_pool,
            tc.tile_pool(name="outp", bufs=4) as out_pool,
            tc.tile_pool(name="ps", bufs=6, space="PSUM") as psum_pool,
        ):
            # scalar ring: x8(0) first (first base matmuls), then w/g8
            # interleaved in 4-chunk pieces so the first MM groups of
            # bt0 fire as early as possible; bias last (first combine
            # needs it ~10us later).
            # two parallel input rings are required for bandwidth:
            # sync carries xb (16MB), scalar carries x8 + consts (10MB).
            # v6-proven order.
            # prewarm the DErf ACT tables with a 1-column dummy op so the
            # ~2.6us of ACT_TABLE_LOADs run during the DMA ramp instead of
            # right before the first real activation.
            warm = const_pool.tile([P, 1], bf16, tag="warm")
            nc.gpsimd.memset(warm[:], 0.0)
            wout = const_pool.tile([P, 1], bf16, tag="wout")
            nc.scalar.activation(wout[:], warm[:], DERF, bias=0.0, scale=1.0)

            # scalar-ring order feeds the PE's fill-phase needs in
            # sequence: x8(0), w -> base(0); x8(1), x8(2) -> base(1..2);
            # g8 arrives by the time the first quantized residuals do.
            NFILL = 3
            x8_pre = []
            x8t0 = x8_pool.tile([P, KO, P], f8, tag="x8")
            nc.scalar.dma_start(x8t0[:], x8_in[0])
            x8_pre.append(x8t0)
            w_sb = const_pool.tile([P, KO, O_SH], f8, tag="w")
            nc.scalar.dma_start(w_sb[:], w_in[:])
            for b in range(1, NFILL):
                t = x8_pool.tile([P, KO, P], f8, tag="x8")
                nc.scalar.dma_start(t[:], x8_in[b])
                x8_pre.append(t)
            if NF > 0:
                g8_sb = const_pool.tile([P, NF, O_SH], f8, tag="g8")
                nc.scalar.dma_start(g8_sb[:], g8_in[:])
            if NF < KO:
                gb_sb = const_pool.tile([P, KO - NF, O_SH], bf16, tag="gb")
                nc.scalar.dma_start(gb_sb[:], gb_in[:])
            bias_sb = const_pool.tile([P, O_SH], f32, tag="bias")
            nc.gpsimd.dma_start(bias_sb[:], bias_in[:])
            fill_state = {}

            def emit_spline_combine(bt, ps, dt_, r8t):
                for j in range(NF // 2):
                    nc.tensor.matmul(
                        ps[:],
                        r8t[:, 2 * j : 2 * j + 2],
                        g8_sb[:, 2 * j : 2 * j + 2],
                        start=False,
                        stop=(NF == KO and j == NF // 2 - 1),
                        perf_mode=DR,
                    )
                for k in range(NF, KO):
                    nc.tensor.matmul(
                        ps[:],
                        dt_[:, k],
                        gb_sb[:, k - NF],
                        start=False,
                        stop=(k == KO - 1),
                    )
                t32 = t32_pool.tile([P, O_SH], f32, tag="t32")
                nc.vector.tensor_tensor(t32[:], ps[:], bias_sb[:], add)
                ot = out_pool.tile([P, O_SH], f16, tag="ot")
                nc.vector.tensor_scalar_mul(ot[:], t32[:], 1.0 / PS_SCALE)
                if bt >= NBT - N_OUT_SYNC:
                    nc.sync.dma_start(out[bt * P : (bt + 1) * P, :], ot[:])
                else:
                    nc.gpsimd.dma_start(out[bt * P : (bt + 1) * P, :], ot[:])

            wsync = const_pool.tile([P, 1], f8, tag="wsync")

            for bt in range(NBT):
                xbt = xb_pool.tile([P, KO, P], bf16, tag="xb")
                nc.sync.dma_start(xbt[:], xb_in[bt])
                if bt == 0:
                    # sequencing fence: a 1-byte/partition SBUF->SBUF DMA
                    # that READS w_sb becomes a w-gated descriptor on the
                    # sync ring, so the xb(1..3) prefetch (not needed
                    # until ~15-20us) yields the shared DMA engines to
                    # the critical w transfer during the ramp. Bytes are
                    # unchanged; only the service order shifts.
                    nc.sync.dma_start(wsync[:], w_sb[:, 0, 0:1])
                if bt < NFILL:
                    x8t = x8_pre[bt]
                else:
                    x8t = x8_pool.tile([P, KO, P], f8, tag="x8")
                    nc.scalar.dma_start(x8t[:], x8_in[bt])

                # basis d = (2/sqrt(pi)) exp(-x^2) and fp8 residual
                # quantize r8 = (d - C)*SB, in half-tiles for fill speed
                dt_ = d_pool.tile([P, KO, P], bf16, tag="d")
                nc.scalar.activation(
                    dt_.rearrange("p a b -> p (a b)"),
                    xbt.rearrange("p a b -> p (a b)"),
                    DERF,
                    bias=0.0,
                    scale=1.0,
                )
                if NF > 0:
                    r8t = r8_pool.tile([P, NF, P], f8, tag="r8")
                    nc.gpsimd.tensor_scalar(
                        r8t.rearrange("p a b -> p (a b)"),
                        dt_.rearrange("p a b -> p (a b)")[:, : NF * P],
                        -CC,
                        SB,
                        op0=add,
                        op1=mult,
                    )

                ps = psum_pool.tile([P, O_SH], f32, tag="ps")
                for j in range(KO // 2):
                    nc.tensor.matmul(
                        ps[:],
                        x8t[:, 2 * j : 2 * j + 2],
                        w_sb[:, 2 * j : 2 * j + 2],
                        start=(j == 0),
                        stop=False,
                        perf_mode=DR,
                    )
                if bt < NFILL:
                    fill_state[bt] = (ps, dt_, r8t if NF > 0 else None)
                    if bt == NFILL - 1:
                        for b2 in range(NFILL):
                            emit_spline_combine(b2, *fill_state.pop(b2))
                else:
                    emit_spline_combine(bt, ps, dt_, r8t if NF > 0 else None)

    nc.compile()
    return nc


def _prep_in_maps(x, w, grid):
    xs_t = [
        np.ascontiguousarray(
            x[r * B_SH : (r + 1) * B_SH, :]
            .T.reshape(KO, P, NBT, P)
            .transpose(2, 1, 0, 3)
        )
        for r in range(R)
    ]
    x8_t = [np.asarray(a * SX, dtype=np.float32).astype(F8) for a in xs_t]
    xb_t = [a.astype(BF16) for a in xs_t]
    w_t = [
        np.ascontiguousarray(
            w[c * O_SH : (c + 1) * O_SH, :].T.reshape(KO, P, O_SH).transpose(1, 0, 2)
            * SW
        ).astype(F8)
        for c in range(C)
    ]

    grid64 = grid.astype(np.float64)
    SPI_H = np.sqrt(np.pi) / 2.0
    g8_t, gb_t, bias_t = [], [], []
    for c in range(C):
        Gp = (grid64[:, c * O_SH : (c + 1) * O_SH, :].sum(-1) * SPI_H)  # (IN_F, O_SH)
        Gp_k = Gp.reshape(KO, P, O_SH)
        g8_t.append(
            np.ascontiguousarray(Gp_k[:NF].transpose(1, 0, 2) * SG)
            .astype(np.float32)
            .astype(F8)
        )
        gb_t.append(
            np.ascontiguousarray(Gp_k[NF:].transpose(1, 0, 2) * PS_SCALE)
            .astype(np.float32)
            .astype(BF16)
        )
        # exact rank-1 mean-split bias, only over the fp8 chunks
        colsum = Gp[: NF * P, :].sum(0) * CC * PS_SCALE
        bias_t.append(
            np.broadcast_to(colsum.astype(np.float32), (P, O_SH)).copy()
        )

    in_maps = []
    for core in range(N_CORES):
        r, c = divmod(core, C)
        im = {
            "x8": x8_t[r],
            "xb": xb_t[r],
            "wt": w_t[c],
            "bias": bias_t[c],
        }
        if NF > 0:
            im["g8"] = g8_t[c]
        if NF < KO:
            im["gb"] = gb_t[c]
        in_maps.append(im)
    return in_maps


def _gather(results):
    out_full = np.empty((BATCH, OUT_F), np.float32)
    for core in range(N_CORES):
        r, c = divmod(core, C)
        out_full[
            r * B_SH : (r + 1) * B_SH, c * O_SH : (c + 1) * O_SH
        ] = results[core]["out"].astype(np.float32)
    return out_full


def get_nc():
    global _cached_nc
    if _cached_nc is None:
        _cached_nc = _build_nc()
    return _cached_nc


def run(x, w, grid, **spmd_kwargs):
    nc = get_nc()
    in_maps = _prep_in_maps(x, w, grid)
    res = run_bass_kernel_spmd(
        nc, in_maps, core_ids=list(range(N_CORES)), **spmd_kwargs
    )
    return _gather(res.results), res


def kernel(x, base_weight, grid):
    x = np.asarray(x, dtype=np.float32)
    base_weight = np.asarray(base_weight, dtype=np.float32)
    grid = np.asarray(grid, dtype=np.float32)
    out, _ = run(x, base_weight, grid)
    return out


# revision 49
# speedup vs baseline: 1.0526x; 1.0238x over previous
"""Trainium2 Bass kernel for nn_KANSplineLayer.

Computes, for x:(8192,2048) f32, base_weight:(2048,2048) f32,
grid:(2048,2048,8) f32:

    base_out   = x @ base_weight.T
    basis      = exp(-(x - grid.mean())**2)
    spline_out = basis @ grid.sum(-1)
    out        = base_out + spline_out          # (8192, 2048) f32

Sharding: 8 cores as 2 batch-groups x 4 out-feature groups; each core
computes a (4096, 512) output tile. 221us (v3 baseline) -> 136us.

Design (single fused pass, BOTH matmuls fp8 DoubleRow = 2x PE rate):
  - Host precomputes Gsum = grid.sum(-1) (f64) and a mean-split rank-1
    bias. With d = (2/sqrt(pi))exp(-x^2) (one Derivative_Erf ACT op per
    batch tile) and C = E[d] = (2/sqrt(pi))/sqrt(3):
        spline = d @ Gp = C*colsum(Gp) + (d - C) @ Gp,
    Gp = (sqrt(pi)/2)*Gsum; colsum is exact (host f64), and only the
    residual r = d - C (RMS 0.34 vs 0.68 for d) passes through fp8.
    That halves BOTH the basis-side and the G-side fp8 quantization
    error: 1.84e-2 measured (vs 3.5e-2 unsplit), under the 2e-2 gate;
    hardware matches the numpy fp8 simulation to 4 digits.
    grid.mean() ~ N(0, 0.1/sqrt(33.5M)) is dropped (~2e-5 rel effect).
  - Both matmuls accumulate into ONE PSUM tile at a common 2^16 scale
    (x*32 @ w*2048 and r*256 @ Gp*256); NF spline chunks run fp8 DR,
    any remainder runs bf16 against Gp*2^16 (NF=16: all fp8).
  - Per 128-row tile: 8+8 DR matmuls (215ns each, PE-bound floor
    110us/core), 1 ACT, 1 gpsimd quantize, vector bias-add + fp16
    cast, out DMA. PE runs gap-free in steady state.

Schedule notes (hard-won; perturbing any of these measured WORSE):
  - Two HWDGE rings in parallel are required for input bandwidth
    (~300GB/s/ring, ~8us startup): sync carries xb whole-tile DMAs
    (+last out tiles), scalar carries x8 + w + g8; outputs ride the
    gpsimd SWDGE queue (an output on a prefetch ring head-of-line-
    blocks it). Whole-tile xb (one DMA, 4KB bursts) replaces the v3-
    inherited halves: -32 issue instructions on the sync engine;
    ties-or-beats halves (first three draws 136.1-136.3us, the
    session's best; run distribution has a heavy ~+3us right tail).
  - DMA_DIRECT2D issue costs ~650ns ON the issuing engine, and the
    ACT tables (2x 1.3us) load lazily - a 1-column dummy Derivative_Erf
    before the const DMAs prewarms them so the first real activation
    isn't stuck behind issue work.
  - The PE clock ramps (~630ns/MM early vs 380ns steady, ~25us); the
    first NFILL=3 tiles emit base-matmul groups immediately (deps:
    x8+w only) and defer their act/quantize-gated spline halves,
    bridging the fill while the act chain catches up. Scalar-ring
    order x8(0), w, x8(1), x8(2), g8 feeds the PE in need order.
  - Pool depths (xb/x8 prefetch 4/6, psum 6) are load-bearing:
    xb bufs=3 costs +24us, single-ring inputs cost +5..20us.

Floor anatomy (all verified by experiment; ~136-138us band, +-1.5us
run noise): 110.1us PE stream (512 DR ops at the exact 215ns rate),
~16us boot + DMA ramp (8us NEFF preamble emitted before ANY user
instruction, then 2.75MB of ramp-critical operands at the ~345GB/s
per-core aggregate - reallocating bytes between rings measured
zero-sum), ~4us act-chain catch-up (ACT engine has no fast dtype
mode; 2.2us/tile is fixed), ~7us tail + exit barriers. Also dead:
PE clock prewarm via dummy matmuls (fill is dependency-bound, the
DVFS gain is illusory), outs 8..31 on the sync ring (ties best run
but higher variance: 140.3us tail draw), psum bufs 8, gpsimd reading
PSUM (BIR-illegal), PSUM scale 2^9 for 1-op fp16 combine (fp8
subnormal operands -> inf on hardware).
"""

import numpy as np
import ml_dtypes

import concourse.bass as bass
import concourse.mybir as mybir
import concourse.tile as tile
from concourse import bacc, bass_isa
from concourse.bass_utils import run_bass_kernel_spmd

P = 128            # SBUF partitions
IN_F = 2048
OUT_F = 2048
GG = 8             # grid last dim (grid_size + spline_order)
BATCH = 8192
R = 2              # batch groups
C = 4              # out-feature groups
N_CORES = 8
B_SH = BATCH // R      # 4096 batch rows per core
O_SH = OUT_F // C      # 512 out features per core
KO = IN_F // P         # 16 contraction chunks
NBT = B_SH // P        # 32 batch tiles per core

NF = 16                # spline chunks in fp8 DoubleRow (rest bf16)
H = KO // 2            # half-tile chunk count
N_OUT_SYNC = 6         # trailing out tiles routed via the sync HWDGE ring

SX = 32.0              # x fp8 scale
SW = 2048.0            # w fp8 scale          (SX*SW = 2^16)
SB = 256.0             # basis-residual fp8 scale
SG = 256.0             # Gp fp8 scale         (SB*SG = 2^16)
PS_SCALE = 2.0 ** 16   # PSUM holds 2^16 * out
SPI2 = 1.1283791670955126   # 2/sqrt(pi): Derivative_Erf(t) = SPI2*exp(-t^2)
CC = SPI2 / 1.7320508075688772  # E[d] for x~N(0,1)

BF16 = ml_dtypes.bfloat16
F8 = ml_dtypes.float8_e4m3

_cached_nc = None


def _build_nc():
    nc = bacc.Bacc(
        "TRN2", target_bir_lowering=False, debug=False, num_devices=N_CORES
    )
    f32 = mybir.dt.float32
    bf16 = mybir.dt.bfloat16
    f16 = mybir.dt.float16
    f8 = mybir.dt.float8e4
    add = mybir.AluOpType.add
    mult = mybir.AluOpType.mult
    DR = mybir.MatmulPerfMode.DoubleRow
    DERF = mybir.ActivationFunctionType.Derivative_Erf

    x8_in = nc.dram_tensor("x8", [NBT, P, KO, P], f8, kind="ExternalInput")
    xb_in = nc.dram_tensor("xb", [NBT, P, KO, P], bf16, kind="ExternalInput")
    w_in = nc.dram_tensor("wt", [P, KO, O_SH], f8, kind="ExternalInput")
    if NF > 0:
        g8_in = nc.dram_tensor("g8", [P, NF, O_SH], f8, kind="ExternalInput")
    if NF < KO:
        gb_in = nc.dram_tensor(
            "gb", [P, KO - NF, O_SH], bf16, kind="ExternalInput"
        )
    bias_in = nc.dram_tensor("bias", [P, O_SH], f32, kind="ExternalInput")
    out = nc.dram_tensor("out", [B_SH, O_SH], f16, kind="ExternalOutput")

    with tile.TileContext(nc, pool_alloc_mode="queue") as tc:
        with (
            tc.tile_pool(name="const", bufs=1) as const_pool,
            tc.tile_pool(name="x8p", bufs=6) as x8_pool,
            tc.tile_pool(name="xbp", bufs=4) as xb_pool,
            tc.tile_pool(name="dp", bufs=4) as d_pool,
            tc.tile_pool(name="r8p", bufs=6) as r8_pool,
            tc.tile_pool(name="t32p", bufs=3) as t32_pool,
            tc.tile_pool(name="outp", bufs=4) as out_pool,
            tc.tile_pool(name="ps", bufs=6, space="PSUM") as psum_pool,
        ):
            # scalar ring: x8(0) first (first base matmuls), then w/g8
            # interleaved in 4-chunk pieces so the first MM groups of
            # bt0 fire as early as possible; bias last (first combine
            # needs it ~10us later).
            # two parallel input rings are required for bandwidth:
            # sync carries xb (16MB), scalar carries x8 + consts (10MB).
            # v6-proven order.
            # prewarm the DErf ACT tables with a 1-column dummy op so the
            # ~2.6us of ACT_TABLE_LOADs run during the DMA ramp instead of
            # right before the first real activation.
            warm = const_pool.tile([P, 1], bf16, tag="warm")
            nc.gpsimd.memset(warm[:], 0.0)
            wout = const_pool.tile([P, 1], bf16, tag="wout")
            nc.scalar.activation(wout[:], warm[:], DERF, bias=0.0, scale=1.0)

            # scalar-ring order feeds the PE's fill-phase needs in
            # sequence: x8(0), w -> base(0); x8(1), x8(2) -> base(1..2);
            # g8 arrives by the time the first quantized residuals do.
            NFILL = 3
            x8_pre = []
            x8t0 = x8_pool.tile([P, KO, P], f8, tag="x8")
            nc.scalar.dma_start(x8t0[:], x8_in[0])
            x8_pre.append(x8t0)
            w_sb = const_pool.tile([P, KO, O_SH], f8, tag="w")
            nc.scalar.dma_start(w_sb[:], w_in[:])
            for b in range(1, NFILL):
                t = x8_pool.tile([P, KO, P], f8, tag="x8")
                nc.scalar.dma_start(t[:], x8_in[b])
                x8_pre.append(t)
            if NF > 0:
                g8_sb = const_pool.tile([P, NF, O_SH], f8, tag="g8")
                nc.scalar.dma_start(g8_sb[:], g8_in[:])
            if NF < KO:
                gb_sb = const_pool.tile([P, KO - NF, O_SH], bf16, tag="gb")
                nc.scalar.dma_start(gb_sb[:], gb_in[:])
            bias_sb = const_pool.tile([P, O_SH], f32, tag="bias")
            nc.gpsimd.dma_start(bias_sb[:], bias_in[:])
            fill_state = {}

            def emit_spline_combine(bt, ps, dt_, r8t):
                for j in range(NF // 2):
                    nc.tensor.matmul(
                        ps[:],
                        r8t[:, 2 * j : 2 * j + 2],
                        g8_sb[:, 2 * j : 2 * j + 2],
                        start=False,
                        stop=(NF == KO and j == NF // 2 - 1),
                        perf_mode=DR,
                    )
                for k in range(NF, KO):
                    nc.tensor.matmul(
                        ps[:],
                        dt_[:, k],
                        gb_sb[:, k - NF],
                        start=False,
                        stop=(k == KO - 1),
                    )
                t32 = t32_pool.tile([P, O_SH], f32, tag="t32")
                nc.vector.tensor_tensor(t32[:], ps[:], bias_sb[:], add)
                ot = out_pool.tile([P, O_SH], f16, tag="ot")
                nc.vector.tensor_scalar_mul(ot[:], t32[:], 1.0 / PS_SCALE)
                if bt >= NBT - N_OUT_SYNC:
                    nc.sync.dma_start(out[bt * P : (bt + 1) * P, :], ot[:])
                else:
                    nc.gpsimd.dma_start(out[bt * P : (bt + 1) * P, :], ot[:])

            for bt in range(NBT):
                xbt = xb_pool.tile([P, KO, P], bf16, tag="xb")
                if 1 <= bt <= 6:
                    # hard WAW fence: a 1-element write that READS w_sb
                    # forces this xb DMA to schedule after w completes
                    # (the scheduler can't hoist past a real dependency).
                    # The sync ring therefore yields the shared DMA
                    # engines to the critical w transfer during the ramp;
                    # the DMA overwrites the element, so no data impact.
                    nc.vector.tensor_scalar_mul(
                        xbt[0:1, 0, 0:1], w_sb[0:1, 0, 0:1], 0.0
                    )
                nc.sync.dma_start(xbt[:], xb_in[bt])
                if bt < NFILL:
                    x8t = x8_pre[bt]
                else:
                    x8t = x8_pool.tile([P, KO, P], f8, tag="x8")
                    nc.scalar.dma_start(x8t[:], x8_in[bt])

                # basis d = (2/sqrt(pi)) exp(-x^2) and fp8 residual
                # quantize r8 = (d - C)*SB, in half-tiles for fill speed
                dt_ = d_pool.tile([P, KO, P], bf16, tag="d")
                nc.scalar.activation(
                    dt_.rearrange("p a b -> p (a b)"),
                    xbt.rearrange("p a b -> p (a b)"),
                    DERF,
                    bias=0.0,
                    scale=1.0,
                )
                if NF > 0:
                    r8t = r8_pool.tile([P, NF, P], f8, tag="r8")
                    nc.gpsimd.tensor_scalar(
                        r8t.rearrange("p a b -> p (a b)"),
                        dt_.rearrange("p a b -> p (a b)")[:, : NF * P],
                        -CC,
                        SB,
                        op0=add,
                        op1=mult,
                    )

                ps = psum_pool.tile([P, O_SH], f32, tag="ps")
                for j in range(KO // 2):
                    nc.tensor.matmul(
                        ps[:],
                        x8t[:, 2 * j : 2 * j + 2],
                        w_sb[:, 2 * j : 2 * j + 2],
                        start=(j == 0),
                        stop=False,
                        perf_mode=DR,
                    )
                if bt < NFILL:
                    fill_state[bt] = (ps, dt_, r8t if NF > 0 else None)
                    if bt == NFILL - 1:
                        for b2 in range(NFILL):
                            emit_spline_combine(b2, *fill_state.pop(b2))
                else:
                    emit_spline_combine(bt, ps, dt_, r8t if NF > 0 else None)

    nc.compile()
    return nc


def _prep_in_maps(x, w, grid):
    xs_t = [
        np.ascontiguousarray(
            x[r * B_SH : (r + 1) * B_SH, :]
            .T.reshape(KO, P, NBT, P)
            .transpose(2, 1, 0, 3)
        )
        for r in range(R)
    ]
    x8_t = [np.asarray(a * SX, dtype=np.float32).astype(F8) for a in xs_t]
    xb_t = [a.astype(BF16) for a in xs_t]
    w_t = [
        np.ascontiguousarray(
            w[c * O_SH : (c + 1) * O_SH, :].T.reshape(KO, P, O_SH).transpose(1, 0, 2)
            * SW
        ).astype(F8)
        for c in range(C)
    ]

    grid64 = grid.astype(np.float64)
    SPI_H = np.sqrt(np.pi) / 2.0
    g8_t, gb_t, bias_t = [], [], []
    for c in range(C):
        Gp = (grid64[:, c * O_SH : (c + 1) * O_SH, :].sum(-1) * SPI_H)  # (IN_F, O_SH)
        Gp_k = Gp.reshape(KO, P, O_SH)
        g8_t.append(
            np.ascontiguousarray(Gp_k[:NF].transpose(1, 0, 2) * SG)
            .astype(np.float32)
            .astype(F8)
        )
        gb_t.append(
            np.ascontiguousarray(Gp_k[NF:].transpose(1, 0, 2) * PS_SCALE)
            .astype(np.float32)
            .astype(BF16)
        )
        # exact rank-1 mean-split bias, only over the fp8 chunks
        colsum = Gp[: NF * P, :].sum(0) * CC * PS_SCALE
        bias_t.append(
            np.broadcast_to(colsum.astype(np.float32), (P, O_SH)).copy()
        )

    in_maps = []
    for core in range(N_CORES):
        r, c = divmod(core, C)
        im = {
            "x8": x8_t[r],
            "xb": xb_t[r],
            "wt": w_t[c],
            "bias": bias_t[c],
        }
        if NF > 0:
            im["g8"] = g8_t[c]
        if NF < KO:
            im["gb"] = gb_t[c]
        in_maps.append(im)
    return in_maps


def _gather(results):
    out_full = np.empty((BATCH, OUT_F), np.float32)
    for core in range(N_CORES):
        r, c = divmod(core, C)
        out_full[
            r * B_SH : (r + 1) * B_SH, c * O_SH : (c + 1) * O_SH
        ] = results[core]["out"].astype(np.float32)
    return out_full


def get_nc():
    global _cached_nc
    if _cached_nc is None:
        _cached_nc = _build_nc()
    return _cached_nc


def run(x, w, grid, **spmd_kwargs):
    nc = get_nc()
    in_maps = _prep_in_maps(x, w, grid)
    res = run_bass_kernel_spmd(
        nc, in_maps, core_ids=list(range(N_CORES)), **spmd_kwargs
    )
    return _gather(res.results), res


def kernel(x, base_weight, grid):
    x = np.asarray(x, dtype=np.float32)
    base_weight = np.asarray(base_weight, dtype=np.float32)
    grid = np.asarray(grid, dtype=np.float32)
    out, _ = run(x, base_weight, grid)
    return out
